# revision 30
# baseline (speedup 1.0000x reference)
"""Trainium2 kernel for nn_CenterDisc (segment_reduce).

Computes: per-class (4 classes) mean of x rows (N=4096 rows of 64x512),
then mean pairwise Frobenius distance between the 4 class centers.

Strategy (data-parallel over N, 8 cores):
  - host: quantize x to fp8 e4m3 (tolerance is 2e-2; measured end-to-end
    quantization error ~3e-4) -> HBM traffic drops 4x vs fp32.
  - host: build one-hot(labels) per shard, pre-tile x into contiguous
    [KC, NFB, 128, FB] DMA blocks (1 MB each).
  - device: per-class partial sums via TensorE matmul
        sums[c, f] = sum_k onehot[k, c] * x[k, f]
    with 4-way PE column tiling: 4 concurrent matmuls at col-groups
    (0,32,64,96), each producing the 4 class sums for a different
    512-feature slice, accumulating the 4 row-chunks of 128 in PSUM.
  - host: add the 8 partial (4, 32768) sums, counts = bincount(labels),
    centers + pairwise norms (tiny) on host.
"""

import numpy as np
import ml_dtypes

import concourse.bass as bass
import concourse.tile as tile
from concourse import bacc, mybir
from concourse.bass_utils import run_bass_kernel_spmd

# Problem shape (hardcoded per contract)
N, C, PDIM = 4096, 64, 512
D = C * PDIM           # 32768 features per row
NCLS = 4               # num classes
CORES = 8
R = N // CORES         # 512 rows per core
KP = 128               # rows per matmul chunk (partition dim)
KC = R // KP           # 4 k-chunks per core
FB = 16384             # fp8 feature columns per DMA block (2 MB DMA)
NFB = D // FB          # 4 DMA blocks per k-chunk
MM = 512               # matmul moving free dim (PSUM bank limit in fp32)
NGRP = 4               # concurrent PE column-group matmuls
SB = NGRP * MM         # features per PSUM super-block (2048)
SBF = FB // SB         # super-blocks per DMA block (4)
NSB = D // SB          # total super-blocks (16)

F8 = mybir.dt.float8e4
NP_F8 = ml_dtypes.float8_e4m3

_NC_CACHE = None


def _build_bass():
    nc = bacc.Bacc()
    x_in = nc.dram_tensor("x", [KC * NFB * KP, FB], F8, kind="ExternalInput")
    # one-hot packed as [p, 4k+c] so a single DMA loads all k-chunks
    oh_in = nc.dram_tensor("onehot", [KP, KC * NCLS], F8,
                           kind="ExternalInput")
    OB = mybir.dt.bfloat16  # partial sums leave as bf16 (error ~1e-4)
    out = nc.dram_tensor("sums", [NGRP * NCLS, NSB * MM], OB,
                         kind="ExternalOutput")

    x_r = x_in[:, :].rearrange("(k fb p) f -> k fb p f", fb=NFB, p=KP)
    NPART = 32 * (NGRP - 1) + NCLS  # highest used psum partition + 1

    with tile.TileContext(nc) as tc:
        with (
            tc.tile_pool(name="ohp", bufs=1) as ohp,
            tc.tile_pool(name="xp", bufs=2) as xp,
            tc.tile_pool(name="op", bufs=1) as op,
            tc.tile_pool(name="pp", bufs=8, space="PSUM") as pp,
        ):
            obuf = op.tile([KP, NSB * MM], OB, tag="ob")
            oht = ohp.tile([KP, KC * NCLS], F8, tag="oh")
            nc.scalar.dma_start(out=oht[:], in_=oh_in[:, :])

            for fb in range(NFB):
                xts = []
                for k in range(KC):
                    xt = xp.tile([KP, FB], F8, tag=f"x{k}")
                    eng = nc.sync if k % 2 == 0 else nc.scalar
                    if fb == NFB - 1 and k == KC - 1:
                        # split the final tile so its quads chase the pieces
                        # instead of waiting for the whole 2 MB to land
                        P4 = FB // 4
                        for q in range(4):
                            e = nc.sync if q % 2 == 0 else nc.scalar
                            e.dma_start(out=xt[:, q * P4:(q + 1) * P4],
                                        in_=x_r[k, fb][:, q * P4:(q + 1) * P4])
                    else:
                        eng.dma_start(out=xt[:], in_=x_r[k, fb])
                    xts.append(xt)
                pss = [pp.tile([KP, MM], mybir.dt.float32, tag="ps",
                               name=f"ps{fb}_{s}")
                       for s in range(SBF)]
                for k in range(KC):
                    for s in range(SBF):
                        for g in range(NGRP):
                            off = s * SB + g * MM
                            nc.tensor.matmul(
                                pss[s][32 * g:32 * g + NCLS, :],
                                oht[:, NCLS * k:NCLS * (k + 1)],
                                xts[k][:, off:off + MM],
                                start=(k == 0),
                                stop=(k == KC - 1),
                                tile_position=(0, 32 * g),
                            )
                        if k == KC - 1:
                            # cast right after this super-block's final quad
                            # so the tail isn't one serial burst of casts
                            sb = fb * SBF + s
                            dst = obuf[0:NPART, sb * MM:(sb + 1) * MM]
                            if fb == NFB - 1 and s % 2 == 1:
                                nc.scalar.copy(out=dst, in_=pss[s][0:NPART, :])
                            else:
                                nc.vector.tensor_copy(out=dst,
                                                      in_=pss[s][0:NPART, :])
                # flush this block's columns: one DMA per class, partition
                # stride 32 picks row c of every quadrant (4 SDMA engines).
                # fb0's wave overlaps the stream; only fb1's wave is tail.
                engs = [nc.sync, nc.scalar, nc.sync, nc.scalar]
                ob_g = obuf[:, :].rearrange("(g r) f -> g r f", r=32)
                cl, ch = fb * SBF * MM, (fb + 1) * SBF * MM
                for c in range(NCLS):
                    engs[c].dma_start(
                        out=out[NGRP * c:NGRP * (c + 1), cl:ch],
                        in_=ob_g[:, c, cl:ch])
    nc.compile()
    return nc


def _get_nc():
    global _NC_CACHE
    if _NC_CACHE is None:
        _NC_CACHE = _build_bass()
    return _NC_CACHE


def _prep_core(xc, lc):
    """xc: (R, D) float32 rows of this core; lc: (R,) labels."""
    xq = np.ascontiguousarray(xc).astype(NP_F8)
    # rows (k*KP + p), features (fb*FB + f) -> [(k fb p), f]
    xq = np.ascontiguousarray(
        xq.reshape(KC, KP, NFB, FB).transpose(0, 2, 1, 3)
    ).reshape(KC * NFB * KP, FB)
    # onehot[p, 4k+c] = (labels[k*KP + p] == c)
    oh = (lc[:, None] == np.arange(NCLS)[None, :]).astype(NP_F8)
    oh = np.ascontiguousarray(
        oh.reshape(KC, KP, NCLS).transpose(1, 0, 2)).reshape(KP, KC * NCLS)
    return {"x": xq, "onehot": oh}


def _unpack_sums(raw):
    # raw: [NCLS*NGRP, NSB*MM] class-major; feature f = sb*SB + g*MM + m
    a = raw.astype(np.float64).reshape(NCLS, NGRP, NSB, MM)
    return a.transpose(0, 2, 1, 3).reshape(NCLS, D)


def _run(x, labels, trace=False, **spmd_kwargs):
    x = np.asarray(x, dtype=np.float32).reshape(N, D)
    labels = np.asarray(labels).astype(np.int64)

    in_maps = [
        _prep_core(x[c * R:(c + 1) * R], labels[c * R:(c + 1) * R])
        for c in range(CORES)
    ]
    nc = _get_nc()
    last_err = None
    for attempt in range(5):
        try:
            br = run_bass_kernel_spmd(nc, in_maps, core_ids=list(range(CORES)),
                                      trace=trace, **spmd_kwargs)
            break
        except Exception as e:  # transient device wedge (NRT_*) — retry
            last_err = e
            import time as _time
            _time.sleep(4.0 * (attempt + 1))
    else:
        raise last_err

    sums = np.zeros((NCLS, D), dtype=np.float64)
    for r in br.results:
        sums += _unpack_sums(r["sums"])
    counts = np.bincount(labels, minlength=NCLS).astype(np.float64)
    safe = np.maximum(counts, 1.0)
    centers = sums / safe[:, None]                         # (NCLS, D)
    diffs = centers[:, None, :] - centers[None, :, :]      # (NCLS, NCLS, D)
    norms = np.sqrt(np.sum(diffs * diffs, axis=-1))        # (NCLS, NCLS)
    iu, ju = np.triu_indices(NCLS, k=1)
    distance = np.sum(norms[iu, ju]) / len(iu)
    return np.asarray(distance, dtype=np.float32), br


def kernel(x, labels):
    result, _ = _run(x, labels, trace=False)
    return result


# revision 31
# speedup vs baseline: 1.0930x; 1.0930x over previous
"""Trainium2 kernel for nn_CenterDisc (segment_reduce).

Computes: per-class (4 classes) mean of x rows (N=4096 rows of 64x512),
then mean pairwise Frobenius distance between the 4 class centers.

Strategy (data-parallel over N, 8 cores):
  - host: quantize x to fp8 e4m3 (tolerance is 2e-2; measured end-to-end
    quantization error ~3e-4) -> HBM traffic drops 4x vs fp32.
  - host: build one-hot(labels) per shard, pre-tile x into contiguous
    [KC, NFB, 128, FB] DMA blocks (1 MB each).
  - device: per-class partial sums via TensorE matmul
        sums[c, f] = sum_k onehot[k, c] * x[k, f]
    with 4-way PE column tiling: 4 concurrent matmuls at col-groups
    (0,32,64,96), each producing the 4 class sums for a different
    512-feature slice, accumulating the 4 row-chunks of 128 in PSUM.
  - host: add the 8 partial (4, 32768) sums, counts = bincount(labels),
    centers + pairwise norms (tiny) on host.
"""

import numpy as np
import ml_dtypes

import concourse.bass as bass
import concourse.tile as tile
from concourse import bacc, mybir
from concourse.bass_utils import run_bass_kernel_spmd

# Problem shape (hardcoded per contract)
N, C, PDIM = 4096, 64, 512
D = C * PDIM           # 32768 features per row
NCLS = 4               # num classes
CORES = 8
R = N // CORES         # 512 rows per core
KP = 128               # rows per matmul chunk (partition dim)
KC = R // KP           # 4 k-chunks per core
FB = 16384             # fp8 feature columns per DMA block (2 MB DMA)
NFB = D // FB          # 4 DMA blocks per k-chunk
MM = 512               # matmul moving free dim (PSUM bank limit in fp32)
NGRP = 4               # concurrent PE column-group matmuls
SB = NGRP * MM         # features per PSUM super-block (2048)
SBF = FB // SB         # super-blocks per DMA block (4)
NSB = D // SB          # total super-blocks (16)

F8 = mybir.dt.float8e4
NP_F8 = ml_dtypes.float8_e4m3

_NC_CACHE = None


def _build_bass():
    nc = bacc.Bacc()
    x_in = nc.dram_tensor("x", [KC * NFB * KP, FB], F8, kind="ExternalInput")
    # one-hot packed as [p, 4k+c] so a single DMA loads all k-chunks
    oh_in = nc.dram_tensor("onehot", [KP, KC * NCLS], F8,
                           kind="ExternalInput")
    OB = mybir.dt.bfloat16  # partial sums leave as bf16 (error ~1e-4)
    out = nc.dram_tensor("sums", [NGRP * NCLS, NSB * MM], OB,
                         kind="ExternalOutput")

    x_r = x_in[:, :].rearrange("(k fb p) f -> k fb p f", fb=NFB, p=KP)
    NPART = 32 * (NGRP - 1) + NCLS  # highest used psum partition + 1

    with tile.TileContext(nc) as tc:
        with (
            tc.tile_pool(name="ohp", bufs=1) as ohp,
            tc.tile_pool(name="xp", bufs=2) as xp,
            tc.tile_pool(name="op", bufs=1) as op,
            tc.tile_pool(name="pp", bufs=8, space="PSUM") as pp,
        ):
            obuf = op.tile([KP, NSB * MM], OB, tag="ob")
            oht = ohp.tile([KP, KC * NCLS], F8, tag="oh")
            nc.scalar.dma_start(out=oht[:], in_=oh_in[:, :])

            for fb in range(NFB):
                xts = []
                for k in range(KC):
                    xt = xp.tile([KP, FB], F8, tag=f"x{k}")
                    eng = nc.sync if k % 2 == 0 else nc.scalar
                    if fb == NFB - 1 and k == KC - 1:
                        # split the final tile so its quads chase the pieces
                        # instead of waiting for the whole 2 MB to land
                        P4 = FB // 4
                        for q in range(4):
                            e = nc.sync if q % 2 == 0 else nc.scalar
                            e.dma_start(out=xt[:, q * P4:(q + 1) * P4],
                                        in_=x_r[k, fb][:, q * P4:(q + 1) * P4])
                    else:
                        eng.dma_start(out=xt[:], in_=x_r[k, fb])
                    xts.append(xt)
                pss = [pp.tile([KP, MM], mybir.dt.float32, tag="ps",
                               name=f"ps{fb}_{s}")
                       for s in range(SBF)]
                for k in range(KC):
                    for s in range(SBF):
                        for g in range(NGRP):
                            off = s * SB + g * MM
                            nc.tensor.matmul(
                                pss[s][32 * g:32 * g + NCLS, :],
                                oht[:, NCLS * k:NCLS * (k + 1)],
                                xts[k][:, off:off + MM],
                                start=(k == 0),
                                stop=(k == KC - 1),
                                tile_position=(0, 32 * g),
                            )
                        if k == KC - 1:
                            # cast right after this super-block's final quad
                            # so the tail isn't one serial burst of casts
                            sb = fb * SBF + s
                            dst = obuf[0:NPART, sb * MM:(sb + 1) * MM]
                            if fb == NFB - 1 and s % 2 == 1:
                                nc.scalar.copy(out=dst, in_=pss[s][0:NPART, :])
                            else:
                                nc.vector.tensor_copy(out=dst,
                                                      in_=pss[s][0:NPART, :])
            # end-flush: one DMA per class, partition stride 32 picks row c
            # of every quadrant -> each DMA spans 4 SDMA engines
            engs = [nc.sync, nc.scalar, nc.sync, nc.scalar]
            ob_g = obuf[:, :].rearrange("(g r) f -> g r f", r=32)
            for c in range(NCLS):
                engs[c].dma_start(
                    out=out[NGRP * c:NGRP * (c + 1), :],
                    in_=ob_g[:, c, :])
    nc.compile()
    return nc


def _get_nc():
    global _NC_CACHE
    if _NC_CACHE is None:
        _NC_CACHE = _build_bass()
    return _NC_CACHE


def _prep_core(xc, lc):
    """xc: (R, D) float32 rows of this core; lc: (R,) labels."""
    xq = np.ascontiguousarray(xc).astype(NP_F8)
    # rows (k*KP + p), features (fb*FB + f) -> [(k fb p), f]
    xq = np.ascontiguousarray(
        xq.reshape(KC, KP, NFB, FB).transpose(0, 2, 1, 3)
    ).reshape(KC * NFB * KP, FB)
    # onehot[p, 4k+c] = (labels[k*KP + p] == c)
    oh = (lc[:, None] == np.arange(NCLS)[None, :]).astype(NP_F8)
    oh = np.ascontiguousarray(
        oh.reshape(KC, KP, NCLS).transpose(1, 0, 2)).reshape(KP, KC * NCLS)
    return {"x": xq, "onehot": oh}


def _unpack_sums(raw):
    # raw: [NCLS*NGRP, NSB*MM] class-major; feature f = sb*SB + g*MM + m
    a = raw.astype(np.float64).reshape(NCLS, NGRP, NSB, MM)
    return a.transpose(0, 2, 1, 3).reshape(NCLS, D)


def _run(x, labels, trace=False, **spmd_kwargs):
    x = np.asarray(x, dtype=np.float32).reshape(N, D)
    labels = np.asarray(labels).astype(np.int64)

    in_maps = [
        _prep_core(x[c * R:(c + 1) * R], labels[c * R:(c + 1) * R])
        for c in range(CORES)
    ]
    nc = _get_nc()
    last_err = None
    for attempt in range(5):
        try:
            br = run_bass_kernel_spmd(nc, in_maps, core_ids=list(range(CORES)),
                                      trace=trace, **spmd_kwargs)
            break
        except Exception as e:  # transient device wedge (NRT_*) — retry
            last_err = e
            import time as _time
            _time.sleep(4.0 * (attempt + 1))
    else:
        raise last_err

    sums = np.zeros((NCLS, D), dtype=np.float64)
    for r in br.results:
        sums += _unpack_sums(r["sums"])
    counts = np.bincount(labels, minlength=NCLS).astype(np.float64)
    safe = np.maximum(counts, 1.0)
    centers = sums / safe[:, None]                         # (NCLS, D)
    diffs = centers[:, None, :] - centers[None, :, :]      # (NCLS, NCLS, D)
    norms = np.sqrt(np.sum(diffs * diffs, axis=-1))        # (NCLS, NCLS)
    iu, ju = np.triu_indices(NCLS, k=1)
    distance = np.sum(norms[iu, ju]) / len(iu)
    return np.asarray(distance, dtype=np.float32), br


def kernel(x, labels):
    result, _ = _run(x, labels, trace=False)
    return result
